# revision 62
# baseline (speedup 1.0000x reference)
"""Trainium2 Bass kernel for the MHSA bottleneck block.

Contract: kernel(**inputs) takes the FULL unsharded inputs (as produced by
setup_inputs()) and returns the FULL [64, 2048, 14, 14] float32 output.
Internally shards data-parallel over batch: 8 images per NeuronCore, 8 cores.

v3: bf16 datapath (conv1/qkv/scores) + fp8 DoubleRow matmuls (value proj,
attention output, conv3). Attention scores are computed pre-transposed
([m, n] layout, softmax over partitions via ones-matmul + reciprocal +
broadcast matmul), removing PE transposes and PSUM->SBUF shuffles.
The emission order software-pipelines pair p's conv1/qk against pair
p-1's attention/conv3 so the in-order PE queue never drains.
"""
import sys

sys.path.insert(0, '/opt/trn_rl_repo')

import numpy as np
import ml_dtypes

# Problem constants (hardcoded per the harness contract).
B, CIN, P, H, W = 64, 2048, 512, 14, 14
EPS = 1e-5
N = H * W            # 196 pixels
NCORES = 8
BPC = B // NCORES    # 8 images per core
NPAIR = BPC // 2     # 4 image pairs per core
KC1 = CIN // 128     # 16 input-channel chunks for conv1 / output chunks conv3
PC = P // 128        # 4 chunks of the 512-dim
N2 = 2 * N           # 392 = free dim for image-pair matmuls

# m chunking of the 196-pixel dim: 128 + 68
NCHUNKS = [(0, 128), (128, 68)]

_CACHE = {}


def _build():
    import concourse.bass as bass  # noqa: F401
    import concourse.mybir as mybir
    import concourse.tile as tile
    from concourse import bacc

    f32 = mybir.dt.float32
    bf16 = mybir.dt.bfloat16
    f8 = mybir.dt.float8e4
    DR = mybir.MatmulPerfMode.DoubleRow
    add_op = mybir.AluOpType.add

    nc = bacc.Bacc(None, target_bir_lowering=False, debug=False)

    x_d = nc.declare_dram_parameter("x", [KC1, 128, BPC * N], bf16, isOutput=False)
    w1t_d = nc.declare_dram_parameter("w1t", [KC1, 128, P], bf16, isOutput=False)
    wqkt_d = nc.declare_dram_parameter("wqkt", [PC, 128, 2 * P], bf16, isOutput=False)
    wvt_d = nc.declare_dram_parameter("wvt", [PC, 128, P], f8, isOutput=False)
    w3t_d = nc.declare_dram_parameter("w3t", [PC, 128, CIN], f8, isOutput=False)
    pos_d = nc.declare_dram_parameter("pos", [PC, 128, N], bf16, isOutput=False)
    # packed per-channel bias/scale vectors: t1 | s2 | t2 | t3
    tb_d = nc.declare_dram_parameter("tb", [128, 3 * PC + KC1], f32,
                                     isOutput=False)
    y_d = nc.declare_dram_parameter("y", [KC1, 128, BPC * N], bf16, isOutput=True)

    with tile.TileContext(nc) as tc:
        with (
            tc.tile_pool(name="const", bufs=1) as const,
            tc.tile_pool(name="xp", bufs=3) as xp,
            tc.tile_pool(name="h1p", bufs=2) as h1p,
            tc.tile_pool(name="qkp", bufs=2) as qkp,
            tc.tile_pool(name="h2p", bufs=2) as h2p,
            tc.tile_pool(name="attp", bufs=2) as attp,
            tc.tile_pool(name="outp", bufs=4) as outp,
            tc.tile_pool(name="ps_mm", bufs=5, space="PSUM") as ps_mm,
            tc.tile_pool(name="ps_sm", bufs=3, space="PSUM") as ps_sm,
        ):
            Exp = mybir.ActivationFunctionType.Exp
            Relu = mybir.ActivationFunctionType.Relu
            Copy = mybir.ActivationFunctionType.Copy

            S = [dict() for _ in range(NPAIR)]

            def emit_x_dma(p, chunks=range(4)):
                if 'x' in S[p]:
                    x_t = S[p]['x']
                else:
                    x_t = xp.tile([128, KC1, N2], bf16, name=f"x_{p}",
                                  tag="x")
                    S[p]['x'] = x_t
                nsl = slice(p * N2, (p + 1) * N2)
                for kq in chunks:
                    nc.sync.dma_start(
                        out=x_t[:, kq * 4:(kq + 1) * 4, :],
                        in_=x_d[kq * 4:(kq + 1) * 4, :, nsl].rearrange(
                            "k p n -> p k n"),
                    )

            # Startup order on the Sync DMA queue: first kc-chunks of x0 and
            # w1t plus the packed biases (everything conv1 oc-block 0 needs to
            # begin), then the remaining chunks. Subtile deps let the first
            # matmuls start as soon as chunk 0 lands. The other weights are
            # emitted mid-prologue so they can't gate the first matmuls.
            # w1t is split into four separate tiles: DMA-completion waits are
            # tile-granular, so the first conv1 matmuls only gate on the
            # first quarter. All startup transfers share one hardware DMA
            # path (~300 GB/s), so strict priority order on a single queue
            # beats spreading across queues: the 0.9 MB the first matmuls
            # need goes first, the rest streams under compute.
            w1q = [const.tile([128, 4, P], bf16, name=f"w1q{k}")
                   for k in range(4)]

            def emit_w1_dma(chunks):
                for kq in chunks:
                    nc.sync.dma_start(
                        out=w1q[kq],
                        in_=w1t_d[kq * 4:(kq + 1) * 4, :, :].rearrange(
                            "k p o -> p k o"))

            emit_x_dma(0, chunks=[0])
            emit_w1_dma([0])
            tb = const.tile([128, 3 * PC + KC1], f32)
            nc.sync.dma_start(out=tb, in_=tb_d[:, :])
            t1 = tb[:, 0:PC]
            s2 = tb[:, PC:2 * PC]
            t2 = tb[:, 2 * PC:3 * PC]
            t3 = tb[:, 3 * PC:3 * PC + KC1]
            for kq in (1, 2, 3):
                emit_x_dma(0, chunks=[kq])
                emit_w1_dma([kq])
            wqk_h = [const.tile([128, PC, P], bf16, name=f"wqk{h}")
                     for h in range(2)]
            wvt = const.tile([128, PC, P], f8)
            pos = const.tile([128, PC, N], bf16)
            w3t = const.tile([128, PC, CIN], f8)
            ones_sb = const.tile([128, 128], bf16)
            nc.gpsimd.memset(ones_sb, 1.0)
            from concourse.masks import make_identity
            identb = const.tile([128, 128], bf16)
            make_identity(nc, identb)

            def emit_late_weights():
                for h in range(2):
                    nc.sync.dma_start(
                        out=wqk_h[h],
                        in_=wqkt_d[:, :, h * P:(h + 1) * P].rearrange(
                            "k p o -> p k o"))
                nc.sync.dma_start(out=wvt, in_=_rearr(wvt_d))
                nc.sync.dma_start(out=pos, in_=_rearr(pos_d))
                nc.sync.dma_start(out=w3t, in_=_rearr(w3t_d))

            # ---------------- block emitters ----------------
            def conv1_block(p, oc):
                Sp = S[p]
                if oc == 0:
                    Sp['h1'] = h1p.tile([128, PC, N2], bf16,
                                        name=f"h1_{p}", tag="h1")
                    Sp['h18'] = h1p.tile([128, PC, 2, 256], f8,
                                         name=f"h18_{p}", tag="h18")
                    nc.gpsimd.memset(Sp['h18'][:, :, :, N:], 0.0)
                cps = ps_mm.tile([128, 512], f32, name="cps", tag="mm")
                for kc in range(KC1):
                    nc.tensor.matmul(
                        cps[:, :N2],
                        w1q[kc // 4][:, kc % 4, oc * 128:(oc + 1) * 128],
                        Sp['x'][:, kc, :],
                        start=(kc == 0), stop=(kc == KC1 - 1),
                    )
                nc.scalar.activation(Sp['h1'][:, oc, :], cps[:, :N2], Relu,
                                     bias=t1[:, oc:oc + 1])
                nc.gpsimd.tensor_copy(
                    Sp['h18'][:, oc, :, :N],
                    Sp['h1'][:, oc, :].rearrange("p (j n) -> p j n", j=2))

            def qk_block(p, oc):
                Sp = S[p]
                if oc == 0:
                    Sp['q'] = qkp.tile([128, PC, 2, N], bf16,
                                       name=f"q_{p}", tag="q")
                    Sp['k'] = qkp.tile([128, PC, 2, N], bf16,
                                       name=f"k_{p}", tag="k")
                qps = ps_mm.tile([128, 512], f32, name="qps", tag="mm")
                for pc in range(PC):
                    nc.tensor.matmul(
                        qps[:, :N2],
                        wqk_h[oc // PC][:, pc, (oc % PC) * 128:
                                        (oc % PC + 1) * 128],
                        Sp['h1'][:, pc, :],
                        start=(pc == 0), stop=(pc == PC - 1),
                    )
                dst = Sp['q'] if oc < PC else Sp['k']
                c4 = oc % PC
                if oc % 2 == 0:
                    nc.scalar.activation(
                        dst[:, c4, :, :],
                        qps[:, :N2].rearrange("p (j n) -> p j n", j=2), Copy)
                else:
                    nc.vector.tensor_copy(
                        dst[:, c4, :, :],
                        qps[:, :N2].rearrange("p (j n) -> p j n", j=2))

            def vT_block(p, j):
                Sp = S[p]
                vT = attp.tile([128, 2, P], f8, name=f"vT_{p}_{j}", tag="vT")
                expT = attp.tile([128, 2, N], bf16, name=f"eT_{p}_{j}",
                                 tag="expT")
                attnT = attp.tile([128, 2, N], f8, name=f"aT_{p}_{j}",
                                  tag="attnT")
                Sp[f'vT{j}'], Sp[f'expT{j}'], Sp[f'attnT{j}'] = vT, expT, attnT
                # rows 68.. of the second m-chunk stay zero (m=196..255 pad)
                nc.gpsimd.memset(attnT[64:128, 1, :], 0.0)
                for mi in range(2):
                    vps = ps_mm.tile([128, 512], f32, name="vps", tag="mm")
                    for i in range(2):
                        nc.tensor.matmul(
                            vps[:, :],
                            Sp['h18'][:, 2 * i:2 * i + 2, j,
                                      mi * 128:(mi + 1) * 128],
                            wvt[:, 2 * i:2 * i + 2, :],
                            start=(i == 0), stop=(i == 1),
                            perf_mode=DR,
                        )
                    nc.vector.tensor_copy(vT[:, mi, :], vps[:, :])

            def sT_block(p, j, mi):
                Sp = S[p]
                m0, msz = NCHUNKS[mi]
                q, k = Sp['q'], Sp['k']
                lps = ps_sm.tile([128, 256], f32, name="lps", tag="small")
                # scores transposed: sT[m, n] = sum_c k[c,m] q[c,n]
                #                             + sum_c q[c,m] pos[c,n]
                for pc in range(PC):
                    nc.tensor.matmul(
                        lps[:msz, :N],
                        k[:, pc, j, m0:m0 + msz],
                        q[:, pc, j, :],
                        start=(pc == 0), stop=False,
                    )
                for pc in range(PC):
                    nc.tensor.matmul(
                        lps[:msz, :N],
                        q[:, pc, j, m0:m0 + msz],
                        pos[:, pc, :],
                        start=False, stop=(pc == PC - 1),
                    )
                # exp (no max subtraction: logits O(40) max, finite in fp32,
                # and bf16 holds e^40 fine)
                nc.scalar.activation(Sp[f'expT{j}'][:msz, mi, :],
                                     lps[:msz, :N], Exp)

            def softsum_block(p, j):
                Sp = S[p]
                expT = Sp[f'expT{j}']
                spsum = ps_sm.tile([1, 256], f32, name="spsum", tag="small")
                for mi, (m0, msz) in enumerate(NCHUNKS):
                    nc.tensor.matmul(
                        spsum[:1, :N],
                        ones_sb[:msz, 0:1],
                        expT[:msz, mi, :],
                        start=(mi == 0), stop=(mi == 1),
                    )
                Sp[f'spsum{j}'] = spsum

            def softnorm_block(p, j):
                Sp = S[p]
                rinv32 = attp.tile([1, N], f32, name=f"rinv32_{p}_{j}",
                                   tag="rinv32")
                nc.vector.reciprocal_approx_fast(rinv32[:1, :],
                                                 Sp[f'spsum{j}'][:1, :N])
                rinv = attp.tile([1, N], bf16, name=f"rinv_{p}_{j}",
                                 tag="rinv")
                nc.vector.tensor_copy(rinv[:1, :], rinv32[:1, :])
                rps = ps_sm.tile([128, 256], f32, name="rps", tag="small")
                nc.tensor.matmul(rps[:, :N], ones_sb[0:1, :], rinv[:1, :],
                                 start=True, stop=True)
                expT, attnT = Sp[f'expT{j}'], Sp[f'attnT{j}']
                for mi, (m0, msz) in enumerate(NCHUNKS):
                    nc.vector.tensor_mul(attnT[:msz, mi, :],
                                         expT[:msz, mi, :], rps[:msz, :N])

            def aout_block(p, j):
                Sp = S[p]
                if j == 0:
                    Sp['h2'] = h2p.tile([128, PC, 2, N], f8,
                                        name=f"h2_{p}", tag="h2")
                vT, attnT = Sp[f'vT{j}'], Sp[f'attnT{j}']
                for c4 in range(PC):
                    aps = ps_sm.tile([128, 256], f32, name="aps", tag="small")
                    nc.tensor.matmul(
                        aps[:, :N],
                        vT[:, :, c4 * 128:(c4 + 1) * 128],
                        attnT[:, :, :],
                        start=True, stop=True,
                        perf_mode=DR,
                    )
                    nc.scalar.activation(Sp['h2'][:, c4, j, :], aps[:, :N],
                                         Relu, bias=t2[:, c4:c4 + 1],
                                         scale=s2[:, c4:c4 + 1])

            def conv3_block(p, k4, final=False):
                Sp = S[p]
                nsl = slice(p * N2, (p + 1) * N2)
                y_sb = outp.tile([128, 4, N2], bf16, name="y_sb", tag="y_sb")
                for i4, oc in enumerate(range(4 * k4, 4 * k4 + 4)):
                    # Final pair only: fold the residual into PSUM with an
                    # identity matmul (PE is idle afterwards), halving the
                    # epilogue's vector/scalar drain. Mid-kernel this loses:
                    # the identity load serializes against the dual-fp8
                    # weight loads.
                    pe_resid = final
                    ops = ps_mm.tile([128, 512], f32, name="ops", tag="mm")
                    for ch in range(2):
                        nc.tensor.matmul(
                            ops[:, :N2],
                            w3t[:, 2 * ch:2 * ch + 2,
                                oc * 128:(oc + 1) * 128],
                            Sp['h2'][:, 2 * ch:2 * ch + 2, :, :],
                            start=(ch == 0), stop=(ch == 1) and not pe_resid,
                            perf_mode=DR,
                        )
                    if pe_resid:
                        nc.tensor.matmul(ops[:, :N2], identb[:, :],
                                         Sp['x'][:, oc, :],
                                         start=False, stop=True)
                        if final and oc % 2 == 1:
                            nc.vector.tensor_scalar(
                                y_sb[:, i4, :], ops[:, :N2],
                                t3[:, oc:oc + 1], 0.0,
                                op0=add_op, op1=mybir.AluOpType.max)
                        else:
                            nc.scalar.activation(y_sb[:, i4, :], ops[:, :N2],
                                                 Relu, bias=t3[:, oc:oc + 1])
                    else:
                        tmp = outp.tile([128, N2], bf16, name="tmp",
                                        tag="tmp")
                        nc.vector.scalar_tensor_tensor(
                            tmp, ops[:, :N2], t3[:, oc:oc + 1],
                            Sp['x'][:, oc, :], op0=add_op, op1=add_op)
                        if oc % 2 == 0:
                            nc.scalar.activation(y_sb[:, i4, :], tmp, Relu)
                        else:
                            nc.vector.tensor_scalar_max(y_sb[:, i4, :], tmp,
                                                        0.0)
                if final:
                    # drain the epilogue per-oc across both DMA queues so the
                    # last stores overlap the remaining relus
                    for i4 in range(4):
                        eng = nc.sync if i4 % 2 == 0 else nc.scalar
                        eng.dma_start(out=y_d[4 * k4 + i4, :, nsl],
                                      in_=y_sb[:, i4, :])
                else:
                    nc.sync.dma_start(
                        out=y_d[4 * k4:4 * k4 + 4, :, nsl].rearrange(
                            "k p n -> p k n"),
                        in_=y_sb[:, :, :])

            # ---------------- pipeline driver ----------------
            def A_blocks(p):
                return ([lambda p=p, oc=oc: conv1_block(p, oc)
                         for oc in range(PC)] +
                        [lambda p=p, oc=oc: qk_block(p, oc)
                         for oc in range(2 * PC)])

            def B_blocks(p, final=False):
                # the two images' chains are independent; interleaving them
                # j0/j1 gives every dependent step a full block of slack
                out = []
                for step in (vT_block,
                             lambda p, j: sT_block(p, j, 0),
                             lambda p, j: sT_block(p, j, 1),
                             softsum_block, softnorm_block, aout_block):
                    for j in range(2):
                        out.append(lambda p=p, j=j, s=step: s(p, j))
                out += [lambda p=p, k=k: conv3_block(p, k, final)
                        for k in range(4)]
                return out

            def interleave(Bl, Al):
                nB, nA = len(Bl), len(Al)
                ai = 0
                for bi, b in enumerate(Bl):
                    b()
                    target = ((bi + 1) * nA) // nB
                    while ai < target:
                        Al[ai]()
                        ai += 1
                while ai < nA:
                    Al[ai]()
                    ai += 1

            prevB = None
            for p in range(NPAIR):
                A = A_blocks(p)
                if prevB is None:
                    for idx, a in enumerate(A):
                        a()
                        if idx == 1:
                            emit_late_weights()
                    # pair-1 prefetch stays behind the late weights so wqkt
                    # isn't displaced on the shared DMA path
                    emit_x_dma(1)
                else:
                    # prefetch ahead of this iteration's y stores: those wait
                    # on relus and would otherwise head-block the transfer
                    if p + 1 < NPAIR:
                        emit_x_dma(p + 1)
                    interleave(prevB, A)
                prevB = B_blocks(p, final=(p == NPAIR - 1))
            for b in prevB:
                b()

    nc.compile()
    return nc


def _rearr(d):
    return d[:, :, :].rearrange("k p o -> p k o")


def _prep_inputs(x, w1, g1, b1, m1, v1, wqkv, rel_h, rel_w,
                 g2, b2, m2, v2, w3, g3, b3, m3, v3):
    f = np.float32
    bf = ml_dtypes.bfloat16
    f8 = ml_dtypes.float8_e4m3
    s1 = (g1 / np.sqrt(v1 + EPS)).astype(f)
    t1 = (b1 - m1 * s1).astype(f)
    s2 = (g2 / np.sqrt(v2 + EPS)).astype(f)
    t2 = (b2 - m2 * s2).astype(f)
    s3 = (g3 / np.sqrt(v3 + EPS)).astype(f)
    t3 = (b3 - m3 * s3).astype(f)

    w1p = (w1 * s1[:, None]).astype(f)                    # [512, 2048]
    w1t = np.ascontiguousarray(w1p.T).reshape(KC1, 128, P).astype(bf)
    wqk = wqkv[:2 * P].astype(f)                          # [1024, 512]
    wqkt = np.ascontiguousarray(wqk.T).reshape(PC, 128, 2 * P).astype(bf)
    wv = wqkv[2 * P:].astype(f)                           # [512, 512]
    wvt = np.ascontiguousarray(wv.T).reshape(PC, 128, P).astype(f8)
    w3p = (w3 * s3[:, None]).astype(f)                    # [2048, 512]
    w3t = np.ascontiguousarray(w3p.T).reshape(PC, 128, CIN).astype(f8)
    pos = (rel_h + rel_w).reshape(P, N).astype(f).reshape(PC, 128, N).astype(bf)

    tb = np.concatenate([t1.reshape(PC, 128).T, s2.reshape(PC, 128).T,
                         t2.reshape(PC, 128).T, t3.reshape(KC1, 128).T],
                        axis=1)
    tb = np.ascontiguousarray(tb, f)

    shared = dict(w1t=w1t, wqkt=wqkt, wvt=wvt, w3t=w3t, pos=pos, tb=tb)

    xbf = np.asarray(x, f).astype(bf)
    in_maps = []
    for c in range(NCORES):
        xc = xbf[c * BPC:(c + 1) * BPC].reshape(BPC, KC1, 128, N)
        xc = np.ascontiguousarray(xc.transpose(1, 2, 0, 3)).reshape(
            KC1, 128, BPC * N)
        in_maps.append(dict(shared, x=xc))
    return in_maps


def _run(in_maps, trace=False, tmpdir=None):
    from concourse.bass_utils import run_bass_kernel_spmd
    if "nc" not in _CACHE:
        _CACHE["nc"] = _build()
    nc = _CACHE["nc"]
    return run_bass_kernel_spmd(nc, in_maps, core_ids=list(range(NCORES)),
                                trace=trace, tmpdir=tmpdir)


def _post(res):
    out = np.empty((B, CIN, H, W), np.float32)
    for c in range(NCORES):
        yc = res.results[c]["y"].astype(np.float32).reshape(KC1, 128, BPC, N)
        out[c * BPC:(c + 1) * BPC] = yc.transpose(2, 0, 1, 3).reshape(
            BPC, CIN, H, W)
    return out


def kernel(**inputs):
    in_maps = _prep_inputs(**inputs)
    res = _run(in_maps)
    return _post(res)


# revision 63
# speedup vs baseline: 1.1579x; 1.1579x over previous
"""Trainium2 Bass kernel for the MHSA bottleneck block.

Contract: kernel(**inputs) takes the FULL unsharded inputs (as produced by
setup_inputs()) and returns the FULL [64, 2048, 14, 14] float32 output.
Internally shards data-parallel over batch: 8 images per NeuronCore, 8 cores.

v3: bf16 datapath (conv1/qkv/scores) + fp8 DoubleRow matmuls (value proj,
attention output, conv3). Attention scores are computed pre-transposed
([m, n] layout, softmax over partitions via ones-matmul + reciprocal +
broadcast matmul), removing PE transposes and PSUM->SBUF shuffles.
The emission order software-pipelines pair p's conv1/qk against pair
p-1's attention/conv3 so the in-order PE queue never drains.
"""
import sys

sys.path.insert(0, '/opt/trn_rl_repo')

import numpy as np
import ml_dtypes

# Problem constants (hardcoded per the harness contract).
B, CIN, P, H, W = 64, 2048, 512, 14, 14
EPS = 1e-5
N = H * W            # 196 pixels
NCORES = 8
BPC = B // NCORES    # 8 images per core
NPAIR = BPC // 2     # 4 image pairs per core
KC1 = CIN // 128     # 16 input-channel chunks for conv1 / output chunks conv3
PC = P // 128        # 4 chunks of the 512-dim
N2 = 2 * N           # 392 = free dim for image-pair matmuls

# m chunking of the 196-pixel dim: 128 + 68
NCHUNKS = [(0, 128), (128, 68)]

_CACHE = {}


def _build():
    import concourse.bass as bass  # noqa: F401
    import concourse.mybir as mybir
    import concourse.tile as tile
    from concourse import bacc

    f32 = mybir.dt.float32
    bf16 = mybir.dt.bfloat16
    f8 = mybir.dt.float8e4
    DR = mybir.MatmulPerfMode.DoubleRow
    add_op = mybir.AluOpType.add

    nc = bacc.Bacc(None, target_bir_lowering=False, debug=False)

    x_d = nc.declare_dram_parameter("x", [KC1, 128, BPC * N], bf16, isOutput=False)
    w1t_d = nc.declare_dram_parameter("w1t", [KC1, 128, P], bf16, isOutput=False)
    wqkt_d = nc.declare_dram_parameter("wqkt", [PC, 128, 2 * P], bf16, isOutput=False)
    wvt_d = nc.declare_dram_parameter("wvt", [PC, 128, P], f8, isOutput=False)
    w3t_d = nc.declare_dram_parameter("w3t", [PC, 128, CIN], f8, isOutput=False)
    pos_d = nc.declare_dram_parameter("pos", [PC, 128, N], bf16, isOutput=False)
    # packed per-channel bias/scale vectors: t1 | s2 | t2 | t3
    tb_d = nc.declare_dram_parameter("tb", [128, 3 * PC + KC1], f32,
                                     isOutput=False)
    y_d = nc.declare_dram_parameter("y", [KC1, 128, BPC * N], bf16, isOutput=True)

    with tile.TileContext(nc) as tc:
        with (
            tc.tile_pool(name="const", bufs=1) as const,
            tc.tile_pool(name="xp", bufs=3) as xp,
            tc.tile_pool(name="h1p", bufs=2) as h1p,
            tc.tile_pool(name="qkp", bufs=2) as qkp,
            tc.tile_pool(name="h2p", bufs=2) as h2p,
            tc.tile_pool(name="attp", bufs=2) as attp,
            tc.tile_pool(name="outp", bufs=4) as outp,
            tc.tile_pool(name="ps_mm", bufs=5, space="PSUM") as ps_mm,
            tc.tile_pool(name="ps_sm", bufs=3, space="PSUM") as ps_sm,
        ):
            Exp = mybir.ActivationFunctionType.Exp
            Relu = mybir.ActivationFunctionType.Relu
            Copy = mybir.ActivationFunctionType.Copy

            S = [dict() for _ in range(NPAIR)]

            def emit_x_dma(p, chunks=range(4)):
                if 'x' in S[p]:
                    x_t = S[p]['x']
                else:
                    x_t = xp.tile([128, KC1, N2], bf16, name=f"x_{p}",
                                  tag="x")
                    S[p]['x'] = x_t
                nsl = slice(p * N2, (p + 1) * N2)
                for kq in chunks:
                    nc.sync.dma_start(
                        out=x_t[:, kq * 4:(kq + 1) * 4, :],
                        in_=x_d[kq * 4:(kq + 1) * 4, :, nsl].rearrange(
                            "k p n -> p k n"),
                    )

            # Startup order on the Sync DMA queue: first kc-chunks of x0 and
            # w1t plus the packed biases (everything conv1 oc-block 0 needs to
            # begin), then the remaining chunks. Subtile deps let the first
            # matmuls start as soon as chunk 0 lands. The other weights are
            # emitted mid-prologue so they can't gate the first matmuls.
            # w1t is split into four separate tiles: DMA-completion waits are
            # tile-granular, so the first conv1 matmuls only gate on the
            # first quarter. All startup transfers share one hardware DMA
            # path (~300 GB/s), so strict priority order on a single queue
            # beats spreading across queues: the 0.9 MB the first matmuls
            # need goes first, the rest streams under compute.
            w1q = [const.tile([128, 4, P], bf16, name=f"w1q{k}")
                   for k in range(4)]

            def emit_w1_dma(chunks):
                for kq in chunks:
                    nc.sync.dma_start(
                        out=w1q[kq],
                        in_=w1t_d[kq * 4:(kq + 1) * 4, :, :].rearrange(
                            "k p o -> p k o"))

            emit_x_dma(0, chunks=[0])
            emit_w1_dma([0])
            tb = const.tile([128, 3 * PC + KC1], f32)
            nc.sync.dma_start(out=tb, in_=tb_d[:, :])
            t1 = tb[:, 0:PC]
            s2 = tb[:, PC:2 * PC]
            t2 = tb[:, 2 * PC:3 * PC]
            t3 = tb[:, 3 * PC:3 * PC + KC1]
            for kq in (1, 2, 3):
                emit_x_dma(0, chunks=[kq])
                emit_w1_dma([kq])
            wqkt = const.tile([128, PC, 2 * P], bf16)
            wvt = const.tile([128, PC, P], f8)
            pos = const.tile([128, PC, N], bf16)
            w3t = const.tile([128, PC, CIN], f8)
            ones_sb = const.tile([128, 128], bf16)
            nc.gpsimd.memset(ones_sb, 1.0)
            from concourse.masks import make_identity
            identb = const.tile([128, 128], bf16)
            make_identity(nc, identb)

            def emit_late_weights():
                nc.sync.dma_start(out=wqkt, in_=_rearr(wqkt_d))
                nc.sync.dma_start(out=wvt, in_=_rearr(wvt_d))
                nc.sync.dma_start(out=pos, in_=_rearr(pos_d))
                nc.sync.dma_start(out=w3t, in_=_rearr(w3t_d))

            # ---------------- block emitters ----------------
            def conv1_block(p, oc):
                Sp = S[p]
                if oc == 0:
                    Sp['h1'] = h1p.tile([128, PC, N2], bf16,
                                        name=f"h1_{p}", tag="h1")
                    Sp['h18'] = h1p.tile([128, PC, 2, 256], f8,
                                         name=f"h18_{p}", tag="h18")
                    nc.gpsimd.memset(Sp['h18'][:, :, :, N:], 0.0)
                cps = ps_mm.tile([128, 512], f32, name="cps", tag="mm")
                for kc in range(KC1):
                    nc.tensor.matmul(
                        cps[:, :N2],
                        w1q[kc // 4][:, kc % 4, oc * 128:(oc + 1) * 128],
                        Sp['x'][:, kc, :],
                        start=(kc == 0), stop=(kc == KC1 - 1),
                    )
                nc.scalar.activation(Sp['h1'][:, oc, :], cps[:, :N2], Relu,
                                     bias=t1[:, oc:oc + 1])
                nc.gpsimd.tensor_copy(
                    Sp['h18'][:, oc, :, :N],
                    Sp['h1'][:, oc, :].rearrange("p (j n) -> p j n", j=2))

            def qk_block(p, oc):
                Sp = S[p]
                if oc == 0:
                    Sp['q'] = qkp.tile([128, PC, 2, N], bf16,
                                       name=f"q_{p}", tag="q")
                    Sp['k'] = qkp.tile([128, PC, 2, N], bf16,
                                       name=f"k_{p}", tag="k")
                qps = ps_mm.tile([128, 512], f32, name="qps", tag="mm")
                for pc in range(PC):
                    nc.tensor.matmul(
                        qps[:, :N2],
                        wqkt[:, pc, oc * 128:(oc + 1) * 128],
                        Sp['h1'][:, pc, :],
                        start=(pc == 0), stop=(pc == PC - 1),
                    )
                dst = Sp['q'] if oc < PC else Sp['k']
                c4 = oc % PC
                if oc % 2 == 0:
                    nc.scalar.activation(
                        dst[:, c4, :, :],
                        qps[:, :N2].rearrange("p (j n) -> p j n", j=2), Copy)
                else:
                    nc.vector.tensor_copy(
                        dst[:, c4, :, :],
                        qps[:, :N2].rearrange("p (j n) -> p j n", j=2))

            def vT_block(p, j):
                Sp = S[p]
                vT = attp.tile([128, 2, P], f8, name=f"vT_{p}_{j}", tag="vT")
                expT = attp.tile([128, 2, N], bf16, name=f"eT_{p}_{j}",
                                 tag="expT")
                attnT = attp.tile([128, 2, N], f8, name=f"aT_{p}_{j}",
                                  tag="attnT")
                Sp[f'vT{j}'], Sp[f'expT{j}'], Sp[f'attnT{j}'] = vT, expT, attnT
                # rows 68.. of the second m-chunk stay zero (m=196..255 pad)
                nc.gpsimd.memset(attnT[64:128, 1, :], 0.0)
                for mi in range(2):
                    vps = ps_mm.tile([128, 512], f32, name="vps", tag="mm")
                    for i in range(2):
                        nc.tensor.matmul(
                            vps[:, :],
                            Sp['h18'][:, 2 * i:2 * i + 2, j,
                                      mi * 128:(mi + 1) * 128],
                            wvt[:, 2 * i:2 * i + 2, :],
                            start=(i == 0), stop=(i == 1),
                            perf_mode=DR,
                        )
                    nc.vector.tensor_copy(vT[:, mi, :], vps[:, :])

            def sT_block(p, j, mi):
                Sp = S[p]
                m0, msz = NCHUNKS[mi]
                q, k = Sp['q'], Sp['k']
                lps = ps_sm.tile([128, 256], f32, name="lps", tag="small")
                # scores transposed: sT[m, n] = sum_c k[c,m] q[c,n]
                #                             + sum_c q[c,m] pos[c,n]
                for pc in range(PC):
                    nc.tensor.matmul(
                        lps[:msz, :N],
                        k[:, pc, j, m0:m0 + msz],
                        q[:, pc, j, :],
                        start=(pc == 0), stop=False,
                    )
                for pc in range(PC):
                    nc.tensor.matmul(
                        lps[:msz, :N],
                        q[:, pc, j, m0:m0 + msz],
                        pos[:, pc, :],
                        start=False, stop=(pc == PC - 1),
                    )
                # exp (no max subtraction: logits O(40) max, finite in fp32,
                # and bf16 holds e^40 fine)
                nc.scalar.activation(Sp[f'expT{j}'][:msz, mi, :],
                                     lps[:msz, :N], Exp)

            def softsum_block(p, j):
                Sp = S[p]
                expT = Sp[f'expT{j}']
                spsum = ps_sm.tile([1, 256], f32, name="spsum", tag="small")
                for mi, (m0, msz) in enumerate(NCHUNKS):
                    nc.tensor.matmul(
                        spsum[:1, :N],
                        ones_sb[:msz, 0:1],
                        expT[:msz, mi, :],
                        start=(mi == 0), stop=(mi == 1),
                    )
                Sp[f'spsum{j}'] = spsum

            def softnorm_block(p, j):
                Sp = S[p]
                rinv32 = attp.tile([1, N], f32, name=f"rinv32_{p}_{j}",
                                   tag="rinv32")
                nc.vector.reciprocal_approx_fast(rinv32[:1, :],
                                                 Sp[f'spsum{j}'][:1, :N])
                rinv = attp.tile([1, N], bf16, name=f"rinv_{p}_{j}",
                                 tag="rinv")
                nc.vector.tensor_copy(rinv[:1, :], rinv32[:1, :])
                rps = ps_sm.tile([128, 256], f32, name="rps", tag="small")
                nc.tensor.matmul(rps[:, :N], ones_sb[0:1, :], rinv[:1, :],
                                 start=True, stop=True)
                expT, attnT = Sp[f'expT{j}'], Sp[f'attnT{j}']
                for mi, (m0, msz) in enumerate(NCHUNKS):
                    nc.vector.tensor_mul(attnT[:msz, mi, :],
                                         expT[:msz, mi, :], rps[:msz, :N])

            def aout_block(p, j):
                Sp = S[p]
                if j == 0:
                    Sp['h2'] = h2p.tile([128, PC, 2, N], f8,
                                        name=f"h2_{p}", tag="h2")
                vT, attnT = Sp[f'vT{j}'], Sp[f'attnT{j}']
                for c4 in range(PC):
                    aps = ps_sm.tile([128, 256], f32, name="aps", tag="small")
                    nc.tensor.matmul(
                        aps[:, :N],
                        vT[:, :, c4 * 128:(c4 + 1) * 128],
                        attnT[:, :, :],
                        start=True, stop=True,
                        perf_mode=DR,
                    )
                    nc.scalar.activation(Sp['h2'][:, c4, j, :], aps[:, :N],
                                         Relu, bias=t2[:, c4:c4 + 1],
                                         scale=s2[:, c4:c4 + 1])

            def conv3_block(p, k4, final=False):
                Sp = S[p]
                nsl = slice(p * N2, (p + 1) * N2)
                y_sb = outp.tile([128, 4, N2], bf16, name="y_sb", tag="y_sb")
                for i4, oc in enumerate(range(4 * k4, 4 * k4 + 4)):
                    # Final pair only: fold the residual into PSUM with an
                    # identity matmul (PE is idle afterwards), halving the
                    # epilogue's vector/scalar drain. Mid-kernel this loses:
                    # the identity load serializes against the dual-fp8
                    # weight loads.
                    pe_resid = final
                    ops = ps_mm.tile([128, 512], f32, name="ops", tag="mm")
                    for ch in range(2):
                        nc.tensor.matmul(
                            ops[:, :N2],
                            w3t[:, 2 * ch:2 * ch + 2,
                                oc * 128:(oc + 1) * 128],
                            Sp['h2'][:, 2 * ch:2 * ch + 2, :, :],
                            start=(ch == 0), stop=(ch == 1) and not pe_resid,
                            perf_mode=DR,
                        )
                    if pe_resid:
                        nc.tensor.matmul(ops[:, :N2], identb[:, :],
                                         Sp['x'][:, oc, :],
                                         start=False, stop=True)
                        if final and oc % 2 == 1:
                            nc.vector.tensor_scalar(
                                y_sb[:, i4, :], ops[:, :N2],
                                t3[:, oc:oc + 1], 0.0,
                                op0=add_op, op1=mybir.AluOpType.max)
                        else:
                            nc.scalar.activation(y_sb[:, i4, :], ops[:, :N2],
                                                 Relu, bias=t3[:, oc:oc + 1])
                    else:
                        tmp = outp.tile([128, N2], bf16, name="tmp",
                                        tag="tmp")
                        nc.vector.scalar_tensor_tensor(
                            tmp, ops[:, :N2], t3[:, oc:oc + 1],
                            Sp['x'][:, oc, :], op0=add_op, op1=add_op)
                        if oc % 2 == 0:
                            nc.scalar.activation(y_sb[:, i4, :], tmp, Relu)
                        else:
                            nc.vector.tensor_scalar_max(y_sb[:, i4, :], tmp,
                                                        0.0)
                if final:
                    # drain the epilogue per-oc across both DMA queues so the
                    # last stores overlap the remaining relus
                    for i4 in range(4):
                        eng = nc.sync if i4 % 2 == 0 else nc.scalar
                        eng.dma_start(out=y_d[4 * k4 + i4, :, nsl],
                                      in_=y_sb[:, i4, :])
                else:
                    nc.sync.dma_start(
                        out=y_d[4 * k4:4 * k4 + 4, :, nsl].rearrange(
                            "k p n -> p k n"),
                        in_=y_sb[:, :, :])

            # ---------------- pipeline driver ----------------
            def A_blocks(p):
                return ([lambda p=p, oc=oc: conv1_block(p, oc)
                         for oc in range(PC)] +
                        [lambda p=p, oc=oc: qk_block(p, oc)
                         for oc in range(2 * PC)])

            def B_blocks(p, final=False):
                # the two images' chains are independent; interleaving them
                # j0/j1 gives every dependent step a full block of slack
                out = []
                for step in (vT_block,
                             lambda p, j: sT_block(p, j, 0),
                             lambda p, j: sT_block(p, j, 1),
                             softsum_block, softnorm_block, aout_block):
                    for j in range(2):
                        out.append(lambda p=p, j=j, s=step: s(p, j))
                out += [lambda p=p, k=k: conv3_block(p, k, final)
                        for k in range(4)]
                return out

            def interleave(Bl, Al):
                nB, nA = len(Bl), len(Al)
                ai = 0
                for bi, b in enumerate(Bl):
                    b()
                    target = ((bi + 1) * nA) // nB
                    while ai < target:
                        Al[ai]()
                        ai += 1
                while ai < nA:
                    Al[ai]()
                    ai += 1

            prevB = None
            for p in range(NPAIR):
                A = A_blocks(p)
                if prevB is None:
                    for idx, a in enumerate(A):
                        a()
                        if idx == 1:
                            emit_late_weights()
                    # pair-1 prefetch stays behind the late weights so wqkt
                    # isn't displaced on the shared DMA path
                    emit_x_dma(1)
                else:
                    # prefetch ahead of this iteration's y stores: those wait
                    # on relus and would otherwise head-block the transfer
                    if p + 1 < NPAIR:
                        emit_x_dma(p + 1)
                    interleave(prevB, A)
                prevB = B_blocks(p, final=(p == NPAIR - 1))
            for b in prevB:
                b()

    nc.compile()
    return nc


def _rearr(d):
    return d[:, :, :].rearrange("k p o -> p k o")


def _prep_inputs(x, w1, g1, b1, m1, v1, wqkv, rel_h, rel_w,
                 g2, b2, m2, v2, w3, g3, b3, m3, v3):
    f = np.float32
    bf = ml_dtypes.bfloat16
    f8 = ml_dtypes.float8_e4m3
    s1 = (g1 / np.sqrt(v1 + EPS)).astype(f)
    t1 = (b1 - m1 * s1).astype(f)
    s2 = (g2 / np.sqrt(v2 + EPS)).astype(f)
    t2 = (b2 - m2 * s2).astype(f)
    s3 = (g3 / np.sqrt(v3 + EPS)).astype(f)
    t3 = (b3 - m3 * s3).astype(f)

    w1p = (w1 * s1[:, None]).astype(f)                    # [512, 2048]
    w1t = np.ascontiguousarray(w1p.T).reshape(KC1, 128, P).astype(bf)
    wqk = wqkv[:2 * P].astype(f)                          # [1024, 512]
    wqkt = np.ascontiguousarray(wqk.T).reshape(PC, 128, 2 * P).astype(bf)
    wv = wqkv[2 * P:].astype(f)                           # [512, 512]
    wvt = np.ascontiguousarray(wv.T).reshape(PC, 128, P).astype(f8)
    w3p = (w3 * s3[:, None]).astype(f)                    # [2048, 512]
    w3t = np.ascontiguousarray(w3p.T).reshape(PC, 128, CIN).astype(f8)
    pos = (rel_h + rel_w).reshape(P, N).astype(f).reshape(PC, 128, N).astype(bf)

    tb = np.concatenate([t1.reshape(PC, 128).T, s2.reshape(PC, 128).T,
                         t2.reshape(PC, 128).T, t3.reshape(KC1, 128).T],
                        axis=1)
    tb = np.ascontiguousarray(tb, f)

    shared = dict(w1t=w1t, wqkt=wqkt, wvt=wvt, w3t=w3t, pos=pos, tb=tb)

    xbf = np.asarray(x, f).astype(bf)
    in_maps = []
    for c in range(NCORES):
        xc = xbf[c * BPC:(c + 1) * BPC].reshape(BPC, KC1, 128, N)
        xc = np.ascontiguousarray(xc.transpose(1, 2, 0, 3)).reshape(
            KC1, 128, BPC * N)
        in_maps.append(dict(shared, x=xc))
    return in_maps


def _run(in_maps, trace=False, tmpdir=None):
    from concourse.bass_utils import run_bass_kernel_spmd
    if "nc" not in _CACHE:
        _CACHE["nc"] = _build()
    nc = _CACHE["nc"]
    return run_bass_kernel_spmd(nc, in_maps, core_ids=list(range(NCORES)),
                                trace=trace, tmpdir=tmpdir)


def _post(res):
    out = np.empty((B, CIN, H, W), np.float32)
    for c in range(NCORES):
        yc = res.results[c]["y"].astype(np.float32).reshape(KC1, 128, BPC, N)
        out[c * BPC:(c + 1) * BPC] = yc.transpose(2, 0, 1, 3).reshape(
            BPC, CIN, H, W)
    return out


def kernel(**inputs):
    in_maps = _prep_inputs(**inputs)
    res = _run(in_maps)
    return _post(res)
